# revision 20
# baseline (speedup 1.0000x reference)
"""Boid policy kernel for Trainium2 (8 NeuronCores).

Strategy (v2)
-------------
Host: sort boids into 16 spatial patches (4 x-strips x 4 y-quantiles of
exactly 512 queries). Each core processes 2 patches sequentially. All
coordinates handed to a patch are shifted (mod 1) so the patch is centered
at (0.5, 0.5); positions produced by jax.random.uniform are multiples of
2^-23, so the shift is bit-exact in integer lattice space and unwrapped
diffs equal toroidal diffs for every pair that can interact.

Per patch the host gathers only the candidates within perception reach of
the patch rectangle (~25 blocks of 128 instead of all 64), placing the
candidates within separation reach in the leading blocks.

Device (per patch, j = candidate on partitions, i = query on free axis):
  * separation-reach blocks ("sep blocks") use the exact path:
      dx2 = ACT Square(qx_bcast + (-cx_j)), dy2 likewise,
      d2 = dx2 + dy2 (DVE/GPSIMD column split, bit-exact fp32 add),
      perc/sep masks = DVE is_le (0/1) -- masks match the reference
      bit-for-bit.
  * perception-only blocks compute d2 on the PE: d2 = |u|^2+|v|^2-2u.v as
    a K=14-feature f32r matmul (two-way lattice splits of the coordinates
    and three-way splits of the squared norms are all tf32-exact; measured
    |err| <= 5e-8 for near pairs, harmless at the 0.04 threshold).
    The perception mask is then one elementwise pass, load-balanced across
    ACT (Sign(PERC2-d2) -> +-1, weights pre-halved) and DVE
    ((d2<=PERC2)-0.5 -> +-0.5, full weights); both conventions differ from
    the 0/1 mask by the same constant, fixed up on the host with a
    per-patch correction vector.
  * PE accumulates the 9 perception sums [count, vxhi, vxlo, vyhi, vylo,
    pxhi, pxlo, pyhi, pylo] and 5 separation sums per block into PSUM
    (hi/lo weight splits survive the f32r tfloat32 rounding exactly).

Host epilogue (f64): add mask-convention corrections, recover
sum(mask*diff) = sum(mask*pc) - qc*count (self-pair cancels), subtract
self from count/velocity sums, normalize the three steers, combine with
weights, add noise, clip by norm.
"""

import numpy as np

import concourse.bass as bass
import concourse.bacc as bacc
import concourse.mybir as mybir
from concourse.tile import TileContext
from concourse.bass_utils import run_bass_kernel_spmd

N = 8192
NCORES = 8
NPATCH = 16
C = N // NPATCH  # 512 queries per patch
PERC2 = float(np.float32(0.2**2))
SEP2 = float(np.float32(0.02**2))
EPS = 1e-8
RCULL_P = 0.2 + 1e-3
RCULL_S = 0.02 + 1e-3
SCL = 1 << 23

_CACHE = {}


def _build(cfg):
    """cfg = ((NP0, NS0, A0), (NP1, NS1, A1), CSPL)."""
    slots, CSPL = cfg[:2], cfg[2]
    f32 = mybir.dt.float32
    f32r = mybir.dt.float32r
    AF = mybir.ActivationFunctionType
    ALU = mybir.AluOpType

    nc = bacc.Bacc()
    params = []
    for s, (NP, NS, A) in enumerate(slots):
        NPP = NP - NS
        p = {
            "qxb": nc.declare_dram_parameter(f"qxb{s}", [1, C], f32, isOutput=False),
            "qyb": nc.declare_dram_parameter(f"qyb{s}", [1, C], f32, isOutput=False),
            "jbs": nc.declare_dram_parameter(f"jbs{s}", [128, 2 * NS], f32, isOutput=False),
            "fq": nc.declare_dram_parameter(f"fq{s}", [14, C], f32r, isOutput=False),
            "fc": nc.declare_dram_parameter(f"fc{s}", [14, 128 * NPP], f32r, isOutput=False),
            "jwp": nc.declare_dram_parameter(f"jwp{s}", [128, 9 * NP], f32r, isOutput=False),
            "jws": nc.declare_dram_parameter(f"jws{s}", [128, 5 * NS], f32r, isOutput=False),
        }
        params.append(p)
    cst_h = nc.declare_dram_parameter("cst", [128, 1], f32, isOutput=False)
    outp_h = nc.declare_dram_parameter("outp", [9, 2 * C], f32, isOutput=True)
    outs_h = nc.declare_dram_parameter("outs", [5, 2 * C], f32, isOutput=True)

    with TileContext(nc) as tc:
        with (
            tc.tile_pool(name="const", bufs=1) as cpool,
            tc.tile_pool(name="work", bufs=4) as wpool,
            tc.tile_pool(name="acc", bufs=1, space="PSUM") as apool,
        ):
            cst = cpool.tile([128, 1], f32)
            nc.gpsimd.dma_start(out=cst[:], in_=cst_h[:, :])
            tiles = []
            for s, (NP, NS, A) in enumerate(slots):
                NPP = NP - NS
                t = {}
                t["fq"] = cpool.tile([14, C], f32r, name=f"fq{s}")
                nc.sync.dma_start(out=t["fq"][:], in_=params[s]["fq"][:, :])
                t["fc"] = cpool.tile([14, 128 * NPP], f32r, name=f"fc{s}")
                nc.sync.dma_start(out=t["fc"][:], in_=params[s]["fc"][:, :])
                qxr = cpool.tile([1, C], f32, name=f"qxr{s}")
                nc.gpsimd.dma_start(out=qxr[:], in_=params[s]["qxb"][:, :])
                qyr = cpool.tile([1, C], f32, name=f"qyr{s}")
                nc.gpsimd.dma_start(out=qyr[:], in_=params[s]["qyb"][:, :])
                t["qx"] = cpool.tile([128, C], f32, name=f"qx{s}")
                nc.gpsimd.partition_broadcast(t["qx"][:], qxr[:])
                t["qy"] = cpool.tile([128, C], f32, name=f"qy{s}")
                nc.gpsimd.partition_broadcast(t["qy"][:], qyr[:])
                t["jbs"] = cpool.tile([128, 2 * NS], f32, name=f"jbs{s}")
                nc.gpsimd.dma_start(out=t["jbs"][:], in_=params[s]["jbs"][:, :])
                t["jwp"] = cpool.tile([128, 9 * NP], f32r, name=f"jwp{s}")
                nc.scalar.dma_start(out=t["jwp"][:], in_=params[s]["jwp"][:, :])
                t["jws"] = cpool.tile([128, 5 * NS], f32r, name=f"jws{s}")
                nc.scalar.dma_start(out=t["jws"][:], in_=params[s]["jws"][:, :])
                t["accp"] = apool.tile([9, C], f32, name=f"accp{s}")
                t["accs"] = apool.tile([5, C], f32, name=f"accs{s}")
                tiles.append(t)

            G = 4  # d2p group size (= dp PSUM buffers)
            for s, (NP, NS, A) in enumerate(slots):
                NPP = NP - NS
                t = tiles[s]
                pcount = 0
                scount = 0

                def emit_sep_stage1(k, t=t):
                    dx2 = wpool.tile([128, C], f32, tag="dx2", bufs=2)
                    nc.scalar.activation(
                        out=dx2[:], in_=t["qx"][:], func=AF.Square,
                        bias=t["jbs"][:, 2 * k:2 * k + 1], scale=1.0)
                    dy2 = wpool.tile([128, C], f32, tag="dy2", bufs=2)
                    nc.scalar.activation(
                        out=dy2[:], in_=t["qy"][:], func=AF.Square,
                        bias=t["jbs"][:, 2 * k + 1:2 * k + 2], scale=1.0)
                    d2 = wpool.tile([128, C], f32, tag="d2", bufs=2)
                    if CSPL > 0:
                        nc.vector.tensor_tensor(
                            out=d2[:, 0:CSPL], in0=dx2[:, 0:CSPL],
                            in1=dy2[:, 0:CSPL], op=ALU.add)
                    if CSPL < C:
                        nc.gpsimd.tensor_tensor(
                            out=d2[:, CSPL:C], in0=dx2[:, CSPL:C],
                            in1=dy2[:, CSPL:C], op=ALU.add)
                    pm = wpool.tile([128, C], f32r, tag="pm", bufs=14)
                    nc.vector.tensor_scalar(
                        out=pm[:], in0=d2[:], scalar1=PERC2, scalar2=None,
                        op0=ALU.is_le)
                    sm = wpool.tile([128, C], f32r, tag="sm", bufs=4)
                    nc.vector.tensor_scalar(
                        out=sm[:], in0=d2[:], scalar1=SEP2, scalar2=None,
                        op0=ALU.is_le)
                    return pm, sm

                ngrp = (NPP + G - 1) // G
                groups = [list(range(G * g, min(G * (g + 1), NPP)))
                          for g in range(ngrp)]
                # sep blocks assigned to group boundaries, stage-1 two
                # boundaries before their aggs are emitted
                seps_at = [[] for _ in range(ngrp + 3)]
                for k in range(NS):
                    seps_at[(k * ngrp) // max(NS, 1)].append(k)

                agg_q = []      # aggs to flush at this boundary
                agg_q1 = []     # aggs queued one boundary ago
                for gi in range(ngrp + 3):
                    grp = groups[gi] if gi < ngrp else []
                    # back-to-back d2p matmuls for this group
                    dps = []
                    for jd in grp:
                        dp = apool.tile([128, C], f32, tag="dp", bufs=G)
                        nc.tensor.matmul(
                            out=dp[:],
                            lhsT=t["fc"][:, 128 * jd:128 * (jd + 1)],
                            rhs=t["fq"][:], start=True, stop=True)
                        dps.append(dp)
                    # flush aggs queued two boundaries ago, back-to-back
                    for acc, jw, w, bi, m, st, sp in agg_q:
                        nc.tensor.matmul(
                            out=acc[:], lhsT=jw[:, w * bi:w * (bi + 1)],
                            rhs=m[:], start=st, stop=sp)
                    agg_q = agg_q1
                    agg_q1 = []
                    # masks for this group (ACT/DVE striped by A)
                    for jd, dp in zip(grp, dps):
                        b = NS + jd
                        pm = wpool.tile([128, C], f32r, tag="pm", bufs=14)
                        on_act = (jd * A) // max(NPP, 1) != \
                            ((jd + 1) * A) // max(NPP, 1)
                        if on_act:
                            nc.scalar.activation(
                                out=pm[:], in_=dp[:], func=AF.Sign,
                                bias=cst[:, 0:1], scale=-1.0)
                        else:
                            nc.vector.tensor_scalar(
                                out=pm[:], in0=dp[:], scalar1=PERC2,
                                scalar2=0.5, op0=ALU.is_le, op1=ALU.subtract)
                        agg_q1.append(
                            (t["accp"], t["jwp"], 9, b, pm, pcount == 0,
                             pcount == NP - 1))
                        pcount += 1
                    # sep-block stage-1 work for this boundary
                    for k in seps_at[gi]:
                        pm, sm = emit_sep_stage1(k)
                        agg_q1.append(
                            (t["accp"], t["jwp"], 9, k, pm, pcount == 0,
                             pcount == NP - 1))
                        pcount += 1
                        agg_q1.append(
                            (t["accs"], t["jws"], 5, k, sm, scount == 0,
                             scount == NS - 1))
                        scount += 1
                for acc, jw, w, bi, m, st, sp in agg_q + agg_q1:
                    nc.tensor.matmul(
                        out=acc[:], lhsT=jw[:, w * bi:w * (bi + 1)],
                        rhs=m[:], start=st, stop=sp)

            for s in range(2):
                po = wpool.tile([9, C], f32, tag=f"po{s}", bufs=1)
                nc.scalar.copy(out=po[:], in_=tiles[s]["accp"][:])
                so = wpool.tile([5, C], f32, tag=f"so{s}", bufs=1)
                nc.vector.tensor_copy(out=so[:], in_=tiles[s]["accs"][:])
                nc.sync.dma_start(out=outp_h[:, C * s:C * (s + 1)], in_=po[:])
                nc.sync.dma_start(out=outs_h[:, C * s:C * (s + 1)], in_=so[:])
    nc.finalize()
    return nc


def _get_nc(cfg):
    if cfg not in _CACHE:
        _CACHE[cfg] = _build(cfg)
    return _CACHE[cfg]


def _hilo(v64):
    hi = np.round(v64 * 1024.0) / 1024.0
    lo = (v64 - hi).astype(np.float32)
    return hi.astype(np.float32), lo


def _split2(v64):
    """lattice value |v|<=0.5 -> two tf32-exact f32 parts."""
    a1 = np.round(v64 * 2048.0) / 2048.0
    a2 = v64 - a1
    return a1.astype(np.float32), a2.astype(np.float32)


def _split3(v64):
    """norm in [0,0.5] -> three tf32-exact f32 parts (residual < 2^-35)."""
    n1 = np.round(v64 * 2048.0) / 2048.0
    r = v64 - n1
    n2 = np.round(r * (1 << 23)) / (1 << 23)
    r = r - n2
    n3 = np.round(r * float(1 << 34)) / float(1 << 34)
    return n1.astype(np.float32), n2.astype(np.float32), n3.astype(np.float32)


def _features_q(ux, uy):
    """query features [14, n] from centered coords (f64)."""
    a1x, a2x = _split2(ux)
    a1y, a2y = _split2(uy)
    un1, un2, un3 = _split3(ux * ux + uy * uy)
    one = np.ones_like(a1x)
    return np.stack([un1, one, a1x, a1y,
                     un2, one, a1x, a2x, a1y, a2y,
                     un3, one, a2x, a2y]).astype(np.float32)


def _features_c(vx, vy, pad):
    """candidate features [14, n]; pad entries get d2 += 64."""
    b1x, b2x = _split2(vx)
    b1y, b2y = _split2(vy)
    vn1, vn2, vn3 = _split3(vx * vx + vy * vy)
    vn1 = np.where(pad, np.float32(64.0), vn1).astype(np.float32)
    z = np.zeros_like(b1x)
    for a in (b1x, b2x, b1y, b2y, vn2, vn3):
        np.copyto(a, np.where(pad, z, a))
    one = np.ones_like(b1x)
    return np.stack([one, vn1, -2 * b1x, -2 * b1y,
                     one, vn2, -2 * b2x, -2 * b1x, -2 * b2y, -2 * b1y,
                     one, vn3, -2 * b2x, -2 * b2y]).astype(np.float32)


def _balance(ns_tot, npp_tot):
    """pick (a = #sign-masks on ACT, CSPL = DVE share of sep adds).

    Measured per-512-col instruction costs (us): ACT square/sign ~0.70,
    DVE mask ~0.64. The sep-block d2 add is split DVE/GPSIMD at a fixed
    192/320 to keep the chain latency short; balance only `a`.
    """
    T_SQ, T_SG, T_STT, T_M = 0.70, 0.70, 0.64, 0.64
    cspl = 192
    dve_fix = (npp_tot * T_STT + ns_tot * 2 * T_M
               + ns_tot * (cspl * 0.0011 + 0.06))
    act_fix = ns_tot * 2 * T_SQ
    a = int(round((dve_fix - act_fix) / (T_SG + T_STT)))
    a = max(0, min(npp_tot, a))
    return a, cspl


def _prepare(pos, vel):
    n = pos.shape[0]
    assert n == N, f"expected {N} boids, got {n}"

    # --- 16 quantile patches: 4 x-strips x 4 y-quantiles of C queries ---
    xorder = np.argsort(pos[:, 0], kind="stable")
    psel = []
    for s in range(4):
        strip = xorder[(n // 4) * s:(n // 4) * (s + 1)]
        yord = np.argsort(pos[strip, 1], kind="stable")
        for tq in range(4):
            psel.append(strip[yord[C * tq:C * (tq + 1)]])

    p64x = pos[:, 0].astype(np.float64)
    p64y = pos[:, 1].astype(np.float64)
    kx = np.round(p64x * SCL).astype(np.int64)
    ky = np.round(p64y * SCL).astype(np.int64)
    lattice = bool(
        np.all(kx.astype(np.float64) == p64x * SCL)
        and np.all(ky.astype(np.float64) == p64y * SCL)
        and kx.min() >= 0 and kx.max() < SCL
        and ky.min() >= 0 and ky.max() < SCL
    )
    vx64 = vel[:, 0].astype(np.float64)
    vy64 = vel[:, 1].astype(np.float64)

    patches = []
    for sel in psel:
        cxm = 0.5 * (p64x[sel].min() + p64x[sel].max())
        cym = 0.5 * (p64y[sel].min() + p64y[sel].max())
        hx = 0.5 * (p64x[sel].max() - p64x[sel].min()) + 2.0 / SCL
        hy = 0.5 * (p64y[sel].max() - p64y[sel].min()) + 2.0 / SCL
        assert hx + 0.2 < 0.49 and hy + 0.2 < 0.49, (hx, hy)
        axk = int(round(cxm * SCL))
        ayk = int(round(cym * SCL))
        if lattice:
            sxk = (kx - axk + (SCL >> 1)) % SCL
            syk = (ky - ayk + (SCL >> 1)) % SCL
            cx = (sxk.astype(np.float64) / SCL).astype(np.float32)
            cy = (syk.astype(np.float64) / SCL).astype(np.float32)
        else:  # fallback: tiny (~1e-9) inexactness vs reference wrap
            cx = np.mod(p64x - axk / SCL + 0.5, 1.0).astype(np.float32)
            cy = np.mod(p64y - ayk / SCL + 0.5, 1.0).astype(np.float32)

        c64x = cx.astype(np.float64)
        c64y = cy.astype(np.float64)
        ddx = np.maximum(np.abs(c64x - 0.5) - hx, 0.0)
        ddy = np.maximum(np.abs(c64y - 0.5) - hy, 0.0)
        dd2 = ddx * ddx + ddy * ddy
        is_sep = dd2 <= RCULL_S * RCULL_S
        is_perc = dd2 <= RCULL_P * RCULL_P
        sep_idx = np.nonzero(is_sep)[0]
        po_idx = np.nonzero(is_perc & ~is_sep)[0]
        order = np.concatenate([sep_idx, po_idx])
        npb = (len(order) + 127) // 128
        nsb = (len(sep_idx) + 127) // 128
        patches.append(dict(sel=sel, cx=cx, cy=cy, c64x=c64x, c64y=c64y,
                            order=order, npb=npb, nsb=nsb))

    # --- pair patches into cores: richest with poorest by block count ---
    idx = sorted(range(NPATCH), key=lambda i: -patches[i]["npb"])
    pairs = [(idx[i], idx[NPATCH - 1 - i]) for i in range(NCORES)]
    NP0 = max(patches[a]["npb"] for a, _ in pairs)
    NP1 = max(patches[b]["npb"] for _, b in pairs)
    NS0 = max(patches[a]["nsb"] for a, _ in pairs)
    NS1 = max(patches[b]["nsb"] for _, b in pairs)
    NS0 = min(NS0, NP0)
    NS1 = min(NS1, NP1)
    npp_tot = (NP0 - NS0) + (NP1 - NS1)
    a_tot, cspl = _balance(NS0 + NS1, npp_tot)
    A0 = min(NP0 - NS0, round(a_tot * (NP0 - NS0) / max(npp_tot, 1)))
    A1 = min(NP1 - NS1, a_tot - A0)
    cfg = ((NP0, NS0, A0), (NP1, NS1, A1), cspl)

    in_maps = [dict() for _ in range(NCORES)]
    meta = [[None, None] for _ in range(NCORES)]
    for ci, pair in enumerate(pairs):
        for s, pi in enumerate(pair):
            NP, NS, A = cfg[s]
            NPP = NP - NS
            p = patches[pi]
            sel, cx, cy = p["sel"], p["cx"], p["cy"]
            c64x, c64y, order = p["c64x"], p["c64y"], p["order"]
            qx = cx[sel]
            qy = cy[sel]
            qxb = qx.reshape(1, C).copy()
            qyb = qy.reshape(1, C).copy()
            fq = _features_q(qx.astype(np.float64) - 0.5,
                             qy.astype(np.float64) - 0.5)

            nord = len(order)
            jbs = np.full((128, 2 * NS), -50.0, np.float32)
            jwp = np.zeros((128, 9 * NP), np.float32)
            jws = np.zeros((128, 5 * NS), np.float32)
            fcx = np.zeros(128 * NPP, np.float64)
            fcy = np.zeros(128 * NPP, np.float64)
            fpad = np.ones(128 * NPP, bool)
            corr9 = np.zeros(9, np.float64)
            for b in range(NP):
                lo_i = 128 * b
                if lo_i >= nord:
                    break
                jj = order[lo_i:lo_i + 128]
                m = len(jj)
                vxhi, vxlo = _hilo(vx64[jj])
                vyhi, vylo = _hilo(vy64[jj])
                pxhi, pxlo = _hilo(c64x[jj] - 0.5)
                pyhi, pylo = _hilo(c64y[jj] - 0.5)
                w = np.zeros((128, 9), np.float64)
                w[:m, 0] = 1.0
                w[:m, 1] = vxhi; w[:m, 2] = vxlo
                w[:m, 3] = vyhi; w[:m, 4] = vylo
                w[:m, 5] = pxhi; w[:m, 6] = pxlo
                w[:m, 7] = pyhi; w[:m, 8] = pylo
                if b < NS:
                    jbs[:m, 2 * b] = -cx[jj]
                    jbs[:m, 2 * b + 1] = -cy[jj]
                    jwp[:, 9 * b:9 * (b + 1)] = w
                    jws[:m, 5 * b] = 1.0
                    jws[:m, 5 * b + 1] = pxhi; jws[:m, 5 * b + 2] = pxlo
                    jws[:m, 5 * b + 3] = pyhi; jws[:m, 5 * b + 4] = pylo
                else:
                    jd = b - NS
                    fcx[128 * jd:128 * jd + m] = c64x[jj] - 0.5
                    fcy[128 * jd:128 * jd + m] = c64y[jj] - 0.5
                    fpad[128 * jd:128 * jd + m] = False
                    corr9 += w.sum(axis=0) / 2.0
                    on_act = (jd * A) // max(NPP, 1) != \
                        ((jd + 1) * A) // max(NPP, 1)
                    if on_act:
                        jwp[:, 9 * b:9 * (b + 1)] = w / 2.0
                    else:
                        jwp[:, 9 * b:9 * (b + 1)] = w
            fc = _features_c(fcx, fcy, fpad)

            im = in_maps[ci]
            im[f"qxb{s}"] = qxb
            im[f"qyb{s}"] = qyb
            im[f"jbs{s}"] = jbs
            im[f"fq{s}"] = fq
            im[f"fc{s}"] = fc
            im[f"jwp{s}"] = jwp
            im[f"jws{s}"] = jws
            meta[ci][s] = dict(
                sel=sel,
                qxc=qx.astype(np.float64) - 0.5,
                qyc=qy.astype(np.float64) - 0.5,
                corr9=corr9,
            )
        in_maps[ci]["cst"] = np.full((128, 1), PERC2, np.float32)
    return in_maps, meta, cfg


def kernel(position, velocity, noise, separation_weight, alignment_weight,
           cohesion_weight, noise_scale):
    pos = np.asarray(position, dtype=np.float32)
    vel = np.asarray(velocity, dtype=np.float32)
    noi = np.asarray(noise, dtype=np.float32)
    ws = float(separation_weight)
    wa = float(alignment_weight)
    wc = float(cohesion_weight)
    nsc = float(noise_scale)

    in_maps, meta, cfg = _prepare(pos, vel)
    vx64 = vel[:, 0].astype(np.float64)
    vy64 = vel[:, 1].astype(np.float64)

    nc = _get_nc(cfg)
    res = run_bass_kernel_spmd(nc, in_maps, list(range(NCORES))).results

    out = np.zeros((N, 2), np.float32)
    for ci in range(NCORES):
        for s in range(2):
            md = meta[ci][s]
            sel = md["sel"]
            P = res[ci]["outp"][:, C * s:C * (s + 1)].astype(np.float64)
            S = res[ci]["outs"][:, C * s:C * (s + 1)].astype(np.float64)
            P = P + md["corr9"][:, None]
            cnt_all = P[0]
            svx, svy = P[1] + P[2], P[3] + P[4]
            spx, spy = P[5] + P[6], P[7] + P[8]
            scn = S[0]
            ssx, ssy = S[1] + S[2], S[3] + S[4]
            qxc, qyc = md["qxc"], md["qyc"]

            cnt = cnt_all - 1.0
            vax = (svx - vx64[sel]) / cnt
            vay = (svy - vy64[sel]) / cnt
            dvx = vax - vx64[sel]
            dvy = vay - vy64[sel]
            pax = (spx - qxc * cnt_all) / cnt
            pay = (spy - qyc * cnt_all) / cnt
            sepx = -(ssx - qxc * scn)
            sepy = -(ssy - qyc * scn)

            n1 = np.maximum(np.sqrt(sepx * sepx + sepy * sepy), EPS)
            n2 = np.maximum(np.sqrt(dvx * dvx + dvy * dvy), EPS)
            n3 = np.maximum(np.sqrt(pax * pax + pay * pay), EPS)

            ax = ws * sepx / n1 + wa * dvx / n2 + wc * pax / n3
            ay = ws * sepy / n1 + wa * dvy / n2 + wc * pay / n3
            ax = ax + nsc * noi[sel, 0].astype(np.float64)
            ay = ay + nsc * noi[sel, 1].astype(np.float64)
            nn = np.sqrt(ax * ax + ay * ay)
            f = np.where(nn > 1.0, 1.0 / np.maximum(nn, EPS), 1.0)
            out[sel, 0] = (ax * f).astype(np.float32)
            out[sel, 1] = (ay * f).astype(np.float32)
    return out


def run_with_trace(np_inputs):
    """Debug helper for test.py: run the device program with trace=True and
    return (exec_time_ns, profile_json_path_or_None)."""
    pos = np.asarray(np_inputs["position"], dtype=np.float32)
    vel = np.asarray(np_inputs["velocity"], dtype=np.float32)
    in_maps, _, cfg = _prepare(pos, vel)
    nc = _get_nc(cfg)
    r = run_bass_kernel_spmd(nc, in_maps, list(range(NCORES)), trace=True)
    return getattr(r, "exec_time_ns", None), getattr(r, "profile_json", None)
